# revision 1
# baseline (speedup 1.0000x reference)
"""Pairwise KL divergence kernel for Trainium2, SPMD across 8 NeuronCores.

out[n, m] = sum_d a[n,d]*(log a[n,d] - log b[m,d])
          = ent[n] - (a @ log(b)^T)[n, m],  ent = rowsum(a * log a)

Sharding: a (and output rows) split 8 ways; b replicated.
Per core: a_shard (1024, 64), b (8192, 64) -> out_shard (1024, 8192).

Pipeline per core:
  - load a_shard natural (128p, 8t, 64d); la = Ln(a) [ACT]; prod = a*la [DVE];
    ent[128,8] via per-tile reduce [DVE]; aT via 8 PE transposes.
  - load b natural in 4 chunks; lb = Ln(b) [ACT]; 64 PE transposes -> lbT (64, 8192).
  - GEMM: for each of 8 n-tiles x 16 m-tiles: psum(128,512) = aT_t.T @ lbT[:, m].
    Evacuate 2 banks at a time fused with the entropy term:
    out_sb = -psum + ent (ACT Identity w/ per-partition bias, alternating with
    DVE tensor_scalar) -> 4 MB DMA per n-tile to HBM.
"""

import numpy as np

N, M, D = 8192, 8192, 64
NCORES = 8
NSHARD = N // NCORES          # 1024 rows of a per core
NT = NSHARD // 128            # 8 n-tiles per core
MT = M // 512                 # 16 m-tiles of 512
BT = M // 128                 # 64 b row-tiles to transpose
B_CHUNK = 16                  # b tiles per load chunk (2048 rows)

# matmul operand dtype: "fp32" (safe, 4 cyc/row) or "fp32r" (1 cyc/row)
MM_DTYPE = "fp32"

_CACHE = {}


def _build(mm_dtype):
    from contextlib import ExitStack

    import concourse.bacc as bacc_mod
    import concourse.bass as bass
    import concourse.mybir as mybir
    import concourse.tile as tile
    from concourse.masks import make_identity

    FP32 = mybir.dt.float32
    AF = mybir.ActivationFunctionType
    ALU = mybir.AluOpType
    AX = mybir.AxisListType

    nc = bacc_mod.Bacc()
    a_d = nc.dram_tensor("a", [NSHARD, D], FP32, kind="ExternalInput")
    b_d = nc.dram_tensor("b", [M, D], FP32, kind="ExternalInput")
    out_d = nc.dram_tensor("out", [NSHARD, M], FP32, kind="ExternalOutput")

    # fp32r matmul operands must be *produced* as float32r (the BIR verifier
    # requires the producing instruction to round) — so the aT/lbT staging
    # tiles themselves carry the matmul dtype and the PSUM->SBUF copies cast.
    MMDT = mybir.dt.float32r if mm_dtype == "fp32r" else FP32

    with tile.TileContext(nc) as tc, ExitStack() as ctx:
        consts = ctx.enter_context(tc.tile_pool(name="consts", bufs=1))
        apool = ctx.enter_context(tc.tile_pool(name="apool", bufs=1))
        bpool = ctx.enter_context(tc.tile_pool(name="bpool", bufs=2))
        lbtp = ctx.enter_context(tc.tile_pool(name="lbtp", bufs=1))
        tpsum = ctx.enter_context(tc.tile_pool(name="tpsum", bufs=2, space="PSUM"))
        mmps = ctx.enter_context(tc.tile_pool(name="mmps", bufs=3, space="PSUM"))
        stage = ctx.enter_context(tc.tile_pool(name="stage", bufs=2))

        ident = consts.tile([128, 128], FP32)
        make_identity(nc, ident)
        # Dummy transpose so PE observes the gpsimd (ident) sem here: the
        # matmul/LDW struct only carries ONE sync wait, so later transposes
        # must each need at most one sem (codegen: "Too many sync waits").
        warm = tpsum.tile([128, 128], FP32, tag="tp")
        nc.tensor.transpose(warm, ident, ident)

        # ---------------- a prologue ----------------
        a_nat = apool.tile([128, NT, D], FP32)        # row t*128+p at [p, t, :]
        nc.sync.dma_start(out=a_nat, in_=a_d[:, :].rearrange("(t p) d -> p t d", p=128))
        la = apool.tile([128, NT, D], FP32)
        nc.scalar.activation(la, a_nat, AF.Ln)
        prod = apool.tile([128, NT, D], FP32)
        nc.vector.tensor_mul(prod, a_nat, la)
        ent = apool.tile([128, NT], FP32)
        for t in range(NT):
            nc.vector.reduce_sum(ent[:, t : t + 1], prod[:, t, :], axis=AX.X)
        aT = apool.tile([64, NT, 128], MMDT)          # aT[:, t, :] = a tile t transposed
        for g in range(2):
            tp = tpsum.tile([64, 4, 128], FP32, tag="tp")
            for j in range(4):
                nc.tensor.transpose(tp[:, j], a_nat[:, g * 4 + j, :], ident)
            nc.scalar.copy(aT[:, g * 4 : (g + 1) * 4, :], tp)

        # ---------------- b prologue ----------------
        lbT = lbtp.tile([64, BT, 128], MMDT)          # lbT[:, bt, :] = lb tile bt transposed
        b_r = b_d[:, :].rearrange("(t p) d -> p t d", p=128)
        n_chunks = BT // B_CHUNK
        for h in range(n_chunks):
            b_nat = bpool.tile([128, B_CHUNK, D], FP32, tag="b_nat")
            nc.sync.dma_start(out=b_nat, in_=b_r[:, h * B_CHUNK : (h + 1) * B_CHUNK, :])
            lb = bpool.tile([128, B_CHUNK, D], FP32, tag="lb")
            nc.scalar.activation(lb, b_nat, AF.Ln)
            for gg in range(B_CHUNK // 4):
                bt0 = h * B_CHUNK + gg * 4
                tp = tpsum.tile([64, 4, 128], FP32, tag="tp")
                for j in range(4):
                    nc.tensor.transpose(tp[:, j], lb[:, gg * 4 + j, :], ident)
                nc.scalar.copy(lbT[:, bt0 : bt0 + 4, :], tp)

        # ---------------- main GEMM + fused evac ----------------
        for t in range(NT):
            out_sb = stage.tile([128, MT, 512], FP32, tag="out_sb")
            lhsT = aT[:, t, :]
            ent_t = ent[:, t : t + 1]
            for g in range(MT // 2):
                ps = mmps.tile([128, 2, 512], FP32, tag="ps")
                for j in range(2):
                    mi = g * 2 + j
                    nc.tensor.matmul(
                        ps[:, j],
                        lhsT,
                        lbT[:, mi * 4 : (mi + 1) * 4, :],
                        start=True,
                        stop=True,
                    )
                dst = out_sb[:, g * 2 : (g + 1) * 2, :]
                if g % 2 == 0:
                    nc.scalar.activation(dst, ps, AF.Identity, bias=ent_t, scale=-1.0)
                else:
                    nc.vector.tensor_scalar(dst, ps, -1.0, ent_t, ALU.mult, ALU.add)
            nc.sync.dma_start(
                out=out_d[t * 128 : (t + 1) * 128, :].rearrange(
                    "p (c m) -> p c m", m=512
                ),
                in_=out_sb,
            )
    # bacc lowering: splits multi-sem waits onto event-semaphore/nop
    # instructions (HW allows one sync wait per engine instruction).
    nc.compile()
    return nc


def _run(a, b, trace=False):
    from concourse.bass_utils import run_bass_kernel_spmd

    if MM_DTYPE not in _CACHE:
        _CACHE[MM_DTYPE] = _build(MM_DTYPE)
    nc = _CACHE[MM_DTYPE]
    a = np.ascontiguousarray(np.asarray(a, dtype=np.float32))
    b = np.ascontiguousarray(np.asarray(b, dtype=np.float32))
    in_maps = [
        {"a": a[i * NSHARD : (i + 1) * NSHARD], "b": b} for i in range(NCORES)
    ]
    res = run_bass_kernel_spmd(nc, in_maps, list(range(NCORES)), trace=trace)
    out = np.concatenate([r["out"] for r in res.results], axis=0)
    return out, res


def kernel(a, b):
    out, _ = _run(a, b, trace=False)
    return out



# revision 2
# speedup vs baseline: 2.9321x; 2.9321x over previous
"""Pairwise KL divergence kernel for Trainium2, SPMD across 8 NeuronCores.

out[n, m] = sum_d a[n,d]*(log a[n,d] - log b[m,d])
          = ent[n] - (a @ log(b)^T)[n, m],  ent = rowsum(a * log a)

Sharding: a (and output rows) split 8 ways; b replicated.
Per core: a_shard (1024, 64), b (8192, 64) -> out_shard (1024, 8192).

Pipeline per core (fp16 GEMM operands, fp16 output staging):
  - load a_shard natural (128p, 8t, 64d); la = Ln(a) [ACT]; prod = a*la [DVE];
    ent[128,8] fp32 via per-tile reduce [DVE]; a cast to fp16 [ACT];
    aT fp16 via 8 PE transposes (fp16 ident -> 1 cyc/row).
  - load b natural in 4 chunks; lb = Ln(b) -> fp16 [ACT]; 64 PE transposes
    -> lbT (64, 8192) fp16.
  - GEMM: for each of 8 n-tiles x 16 m-tiles: psum(128,512)fp32 = aT_t.T @ lbT.
    fp16 operands stream at 1 cyc/row (4x the fp32 rate).
    Evacuate 2 banks at a time fused with the entropy term:
    out_sb = -psum + ent, cast to fp16 (ACT Identity w/ per-partition bias,
    alternating with DVE tensor_scalar) -> 2x 1 MB DMA per n-tile to HBM.
  - host upcasts fp16 -> fp32.

Precision: GEMM operand rounding (a, lb -> fp16) and the fp16 output cast
give ~1e-3 max rel err vs the fp32 reference, well under the 2e-2 gate.
"""

import numpy as np

N, M, D = 8192, 8192, 64
NCORES = 8
NSHARD = N // NCORES          # 1024 rows of a per core
NT = NSHARD // 128            # 8 n-tiles per core
MT = M // 512                 # 16 m-tiles of 512
BT = M // 128                 # 64 b row-tiles to transpose
B_CHUNK = 16                  # b tiles per load chunk (2048 rows)

# GEMM operand dtype: fp16/bf16 (1 cyc/row), fp32r (1 cyc/row, fp32 data),
# fp32 (4 cyc/row, exact)
MM_DTYPE = "fp16"
# output staging/DMA dtype: fp16 halves the dominant output traffic
OUT_DTYPE = "fp16"

_CACHE = {}


def _build(mm_dtype, out_dtype):
    from contextlib import ExitStack

    import concourse.bacc as bacc_mod
    import concourse.bass as bass
    import concourse.mybir as mybir
    import concourse.tile as tile
    from concourse.masks import make_identity

    FP32 = mybir.dt.float32
    AF = mybir.ActivationFunctionType
    ALU = mybir.AluOpType
    AX = mybir.AxisListType

    DT_MM = {
        "fp16": mybir.dt.float16,
        "bf16": mybir.dt.bfloat16,
        "fp32": FP32,
        "fp32r": mybir.dt.float32r,
    }[mm_dtype]
    DT_OUT = {"fp16": mybir.dt.float16, "fp32": FP32}[out_dtype]
    # 2-byte operands are cast BEFORE the PE transposes so the transposes
    # stream at 1 cyc/row and the PSUM staging tiles are 2-byte as well.
    # 4-byte operands transpose as fp32 and cast at the PSUM->SBUF copy
    # (fp32r must be produced by a rounding instruction - the copy).
    two_byte = mm_dtype in ("fp16", "bf16")
    TP_DT = DT_MM if two_byte else FP32

    nc = bacc_mod.Bacc()
    a_d = nc.dram_tensor("a", [NSHARD, D], FP32, kind="ExternalInput")
    b_d = nc.dram_tensor("b", [M, D], FP32, kind="ExternalInput")
    out_d = nc.dram_tensor("out", [NSHARD, M], DT_OUT, kind="ExternalOutput")

    with tile.TileContext(nc) as tc, ExitStack() as ctx:
        consts = ctx.enter_context(tc.tile_pool(name="consts", bufs=1))
        apool = ctx.enter_context(tc.tile_pool(name="apool", bufs=1))
        bpool = ctx.enter_context(tc.tile_pool(name="bpool", bufs=2))
        lbtp = ctx.enter_context(tc.tile_pool(name="lbtp", bufs=1))
        tpsum = ctx.enter_context(tc.tile_pool(name="tpsum", bufs=2, space="PSUM"))
        mmps = ctx.enter_context(tc.tile_pool(name="mmps", bufs=3, space="PSUM"))
        stage = ctx.enter_context(tc.tile_pool(name="stage", bufs=2))

        ident = consts.tile([128, 128], TP_DT)
        make_identity(nc, ident)
        # Dummy transpose so PE observes the gpsimd (ident) sem here: the
        # matmul/LDW struct only carries ONE sync wait, so later transposes
        # must each need at most one sem (codegen: "Too many sync waits").
        warm = tpsum.tile([128, 128], TP_DT, tag="tp")
        nc.tensor.transpose(warm, ident, ident)

        # ---------------- a prologue ----------------
        a_nat = apool.tile([128, NT, D], FP32)        # row t*128+p at [p, t, :]
        nc.sync.dma_start(out=a_nat, in_=a_d[:, :].rearrange("(t p) d -> p t d", p=128))
        la = apool.tile([128, NT, D], FP32)
        nc.scalar.activation(la, a_nat, AF.Ln)
        prod = apool.tile([128, NT, D], FP32)
        nc.vector.tensor_mul(prod, a_nat, la)
        ent = apool.tile([128, NT], FP32)
        for t in range(NT):
            nc.vector.reduce_sum(ent[:, t : t + 1], prod[:, t, :], axis=AX.X)
        if two_byte:
            a_mm = apool.tile([128, NT, D], DT_MM)
            nc.scalar.copy(a_mm, a_nat)
        else:
            a_mm = a_nat
        aT = apool.tile([64, NT, 128], DT_MM)         # aT[:, t, :] = a tile t transposed
        for g in range(2):
            tp = tpsum.tile([64, 4, 128], TP_DT, tag="tp")
            for j in range(4):
                nc.tensor.transpose(tp[:, j], a_mm[:, g * 4 + j, :], ident)
            nc.vector.tensor_copy(aT[:, g * 4 : (g + 1) * 4, :], tp)

        # ---------------- b prologue ----------------
        lbT = lbtp.tile([64, BT, 128], DT_MM)         # lbT[:, bt, :] = lb tile bt transposed
        b_r = b_d[:, :].rearrange("(t p) d -> p t d", p=128)
        n_chunks = BT // B_CHUNK
        for h in range(n_chunks):
            b_nat = bpool.tile([128, B_CHUNK, D], FP32, tag="b_nat")
            nc.sync.dma_start(out=b_nat, in_=b_r[:, h * B_CHUNK : (h + 1) * B_CHUNK, :])
            lb = bpool.tile([128, B_CHUNK, D], TP_DT, tag="lb")
            nc.scalar.activation(lb, b_nat, AF.Ln)
            for gg in range(B_CHUNK // 4):
                bt0 = h * B_CHUNK + gg * 4
                tp = tpsum.tile([64, 4, 128], TP_DT, tag="tp")
                for j in range(4):
                    nc.tensor.transpose(tp[:, j], lb[:, gg * 4 + j, :], ident)
                nc.vector.tensor_copy(lbT[:, bt0 : bt0 + 4, :], tp)

        # ---------------- main GEMM + fused evac ----------------
        out_r = out_d[:, :].rearrange("(t p) (c m) -> t p c m", p=128, m=512)
        for t in range(NT):
            out_sb = stage.tile([128, MT, 512], DT_OUT, tag="out_sb")
            lhsT = aT[:, t, :]
            ent_t = ent[:, t : t + 1]
            for g in range(MT // 2):
                ps = mmps.tile([128, 2, 512], FP32, tag="ps")
                for j in range(2):
                    mi = g * 2 + j
                    nc.tensor.matmul(
                        ps[:, j],
                        lhsT,
                        lbT[:, mi * 4 : (mi + 1) * 4, :],
                        start=True,
                        stop=True,
                    )
                dst = out_sb[:, g * 2 : (g + 1) * 2, :]
                if g % 2 == 0:
                    nc.scalar.activation(dst, ps, AF.Identity, bias=ent_t, scale=-1.0)
                else:
                    nc.vector.tensor_scalar(dst, ps, -1.0, ent_t, ALU.mult, ALU.add)
                # stream the output out in half-tile DMAs so the store
                # overlaps the second half's matmuls/evac
                if g == MT // 4 - 1:
                    nc.sync.dma_start(
                        out=out_r[t, :, 0 : MT // 2, :],
                        in_=out_sb[:, 0 : MT // 2, :],
                    )
                elif g == MT // 2 - 1:
                    nc.sync.dma_start(
                        out=out_r[t, :, MT // 2 : MT, :],
                        in_=out_sb[:, MT // 2 : MT, :],
                    )
    # bacc lowering: splits multi-sem waits onto event-semaphore/nop
    # instructions (HW allows one sync wait per engine instruction).
    nc.compile()
    return nc


def _run(a, b, trace=False):
    from concourse.bass_utils import run_bass_kernel_spmd

    key = (MM_DTYPE, OUT_DTYPE)
    if key not in _CACHE:
        _CACHE[key] = _build(*key)
    nc = _CACHE[key]
    a = np.ascontiguousarray(np.asarray(a, dtype=np.float32))
    b = np.ascontiguousarray(np.asarray(b, dtype=np.float32))
    in_maps = [
        {"a": a[i * NSHARD : (i + 1) * NSHARD], "b": b} for i in range(NCORES)
    ]
    res = run_bass_kernel_spmd(nc, in_maps, list(range(NCORES)), trace=trace)
    out = np.concatenate(
        [np.asarray(r["out"], dtype=np.float32) for r in res.results], axis=0
    )
    return out, res


def kernel(a, b):
    out, _ = _run(a, b, trace=False)
    return out


# revision 3
# speedup vs baseline: 2.9966x; 1.0220x over previous
"""Pairwise KL divergence kernel for Trainium2, SPMD across 8 NeuronCores.

out[n, m] = sum_d a[n,d]*(log a[n,d] - log b[m,d])
          = ent[n] - (a @ log(b)^T)[n, m],  ent = rowsum(a * log a)

Sharding: a (and output rows) split 8 ways; b replicated.
Per core: a_shard (1024, 64), b (8192, 64) -> out_shard (1024, 8192).

Pipeline per core (fp16 GEMM operands, fp16 output staging):
  - issue ALL input DMAs up front (a + 4 b chunks, bpool bufs=4).
  - a cast to fp16 on DVE (keeps ACT free for Ln); aT via 8 PE transposes.
  - ent: la = Ln(a) [ACT], prod = a*la [DVE], per-tile reduce [DVE] - only
    needed by the first evac, so it runs after Ln(b0) in the ACT queue.
  - b: lb = Ln(b) -> fp16 [ACT]; 64 PE transposes -> lbT (64, 8192) fp16.
  - GEMM: 8 n-tiles x 16 m-tiles: psum(128,512)fp32 = aT_t.T @ lbT at
    1 cyc/row. Evac 2 banks at a time fused with the entropy term
    (-psum + ent -> fp16), alternating ACT/DVE; half-tile (1 MB) DMAs,
    quarter-tile on the last n-tile to shorten the drain tail.
  - host upcasts fp16 -> fp32.

Precision: fp16 GEMM operands + fp16 output give ~5e-4 max rel err vs the
fp32 reference, well under the 2e-2 gate.
"""

import numpy as np

N, M, D = 8192, 8192, 64
NCORES = 8
NSHARD = N // NCORES          # 1024 rows of a per core
NT = NSHARD // 128            # 8 n-tiles per core
MT = M // 512                 # 16 m-tiles of 512
BT = M // 128                 # 64 b row-tiles to transpose
B_CHUNK = 16                  # b tiles per load chunk (2048 rows)

# GEMM operand dtype: fp16/bf16 (1 cyc/row), fp32r (1 cyc/row, fp32 data),
# fp32 (4 cyc/row, exact)
MM_DTYPE = "fp16"
# output staging/DMA dtype: fp16 halves the dominant output traffic
OUT_DTYPE = "fp16"

_CACHE = {}


def _build(mm_dtype, out_dtype):
    from contextlib import ExitStack

    import concourse.bacc as bacc_mod
    import concourse.bass as bass
    import concourse.mybir as mybir
    import concourse.tile as tile
    from concourse.masks import make_identity

    FP32 = mybir.dt.float32
    AF = mybir.ActivationFunctionType
    ALU = mybir.AluOpType
    AX = mybir.AxisListType

    DT_MM = {
        "fp16": mybir.dt.float16,
        "bf16": mybir.dt.bfloat16,
        "fp32": FP32,
        "fp32r": mybir.dt.float32r,
    }[mm_dtype]
    DT_OUT = {"fp16": mybir.dt.float16, "fp32": FP32}[out_dtype]
    # 2-byte operands are cast BEFORE the PE transposes so the transposes
    # stream at 1 cyc/row and the PSUM staging tiles are 2-byte as well.
    two_byte = mm_dtype in ("fp16", "bf16")
    TP_DT = DT_MM if two_byte else FP32

    nc = bacc_mod.Bacc()
    a_d = nc.dram_tensor("a", [NSHARD, D], FP32, kind="ExternalInput")
    b_d = nc.dram_tensor("b", [M, D], FP32, kind="ExternalInput")
    out_d = nc.dram_tensor("out", [NSHARD, M], DT_OUT, kind="ExternalOutput")

    n_chunks = BT // B_CHUNK

    with tile.TileContext(nc) as tc, ExitStack() as ctx:
        consts = ctx.enter_context(tc.tile_pool(name="consts", bufs=1))
        apool = ctx.enter_context(tc.tile_pool(name="apool", bufs=1))
        bpool = ctx.enter_context(tc.tile_pool(name="bpool", bufs=n_chunks))
        lbtp = ctx.enter_context(tc.tile_pool(name="lbtp", bufs=1))
        tpsum = ctx.enter_context(tc.tile_pool(name="tpsum", bufs=2, space="PSUM"))
        mmps = ctx.enter_context(tc.tile_pool(name="mmps", bufs=3, space="PSUM"))
        stage = ctx.enter_context(tc.tile_pool(name="stage", bufs=2))

        ident = consts.tile([128, 128], TP_DT)
        make_identity(nc, ident)
        # Dummy transpose so PE observes the gpsimd (ident) sem here: the
        # matmul/LDW struct only carries ONE sync wait, so later transposes
        # must each need at most one sem (codegen: "Too many sync waits").
        warm = tpsum.tile([128, 128], TP_DT, tag="tp")
        nc.tensor.transpose(warm, ident, ident)

        # ---------------- input DMAs, all issued up front ----------------
        a_nat = apool.tile([128, NT, D], FP32)        # row t*128+p at [p, t, :]
        nc.sync.dma_start(out=a_nat, in_=a_d[:, :].rearrange("(t p) d -> p t d", p=128))
        b_r = b_d[:, :].rearrange("(t p) d -> p t d", p=128)
        b_nats = []
        for h in range(n_chunks):
            b_nat = bpool.tile([128, B_CHUNK, D], FP32, tag="b_nat")
            nc.sync.dma_start(out=b_nat, in_=b_r[:, h * B_CHUNK : (h + 1) * B_CHUNK, :])
            b_nats.append(b_nat)

        # ---------------- a prologue ----------------
        if two_byte:
            a_mm = apool.tile([128, NT, D], DT_MM)
            nc.vector.tensor_copy(a_mm, a_nat)        # DVE: ACT stays free for Ln
        else:
            a_mm = a_nat
        aT = apool.tile([64, NT, 128], DT_MM)         # aT[:, t, :] = a tile t transposed
        for g in range(2):
            tp = tpsum.tile([64, 4, 128], TP_DT, tag="tp")
            for j in range(4):
                nc.tensor.transpose(tp[:, j], a_mm[:, g * 4 + j, :], ident)
            nc.vector.tensor_copy(aT[:, g * 4 : (g + 1) * 4, :], tp)

        # ---------------- b chunk 0 + entropy chain ----------------
        lbT = lbtp.tile([64, BT, 128], DT_MM)         # lbT[:, bt, :] = lb tile bt transposed

        def b_chunk(h):
            lb = bpool.tile([128, B_CHUNK, D], TP_DT, tag="lb")
            nc.scalar.activation(lb, b_nats[h], AF.Ln)
            for gg in range(B_CHUNK // 4):
                bt0 = h * B_CHUNK + gg * 4
                tp = tpsum.tile([64, 4, 128], TP_DT, tag="tp")
                for j in range(4):
                    nc.tensor.transpose(tp[:, j], lb[:, gg * 4 + j, :], ident)
                nc.vector.tensor_copy(lbT[:, bt0 : bt0 + 4, :], tp)

        b_chunk(0)

        # entropy term (needed by the first evac, after chunk 0 in ACT order)
        la = apool.tile([128, NT, D], FP32)
        nc.scalar.activation(la, a_nat, AF.Ln)
        prod = apool.tile([128, NT, D], FP32)
        nc.vector.tensor_mul(prod, a_nat, la)
        ent = apool.tile([128, NT], FP32)
        for t in range(NT):
            nc.vector.reduce_sum(ent[:, t : t + 1], prod[:, t, :], axis=AX.X)

        for h in range(1, n_chunks):
            b_chunk(h)

        # ---------------- main GEMM + fused evac ----------------
        out_r = out_d[:, :].rearrange("(t p) (c m) -> t p c m", p=128, m=512)
        for t in range(NT):
            out_sb = stage.tile([128, MT, 512], DT_OUT, tag="out_sb")
            lhsT = aT[:, t, :]
            ent_t = ent[:, t : t + 1]
            # quarter-tile DMAs on the last n-tile shorten the drain tail
            dma_q = MT // 4 if t == NT - 1 else MT // 2
            for g in range(MT // 2):
                ps = mmps.tile([128, 2, 512], FP32, tag="ps")
                for j in range(2):
                    mi = g * 2 + j
                    nc.tensor.matmul(
                        ps[:, j],
                        lhsT,
                        lbT[:, mi * 4 : (mi + 1) * 4, :],
                        start=True,
                        stop=True,
                    )
                dst = out_sb[:, g * 2 : (g + 1) * 2, :]
                if g % 2 == 0:
                    nc.scalar.activation(dst, ps, AF.Identity, bias=ent_t, scale=-1.0)
                else:
                    nc.vector.tensor_scalar(dst, ps, -1.0, ent_t, ALU.mult, ALU.add)
                done = (g + 1) * 2
                if done % dma_q == 0:
                    c0 = done - dma_q
                    nc.sync.dma_start(
                        out=out_r[t, :, c0:done, :],
                        in_=out_sb[:, c0:done, :],
                    )
    # bacc lowering: splits multi-sem waits onto event-semaphore/nop
    # instructions (HW allows one sync wait per engine instruction).
    nc.compile()
    return nc


def _run(a, b, trace=False):
    from concourse.bass_utils import run_bass_kernel_spmd

    key = (MM_DTYPE, OUT_DTYPE)
    if key not in _CACHE:
        _CACHE[key] = _build(*key)
    nc = _CACHE[key]
    a = np.ascontiguousarray(np.asarray(a, dtype=np.float32))
    b = np.ascontiguousarray(np.asarray(b, dtype=np.float32))
    in_maps = [
        {"a": a[i * NSHARD : (i + 1) * NSHARD], "b": b} for i in range(NCORES)
    ]
    res = run_bass_kernel_spmd(nc, in_maps, list(range(NCORES)), trace=trace)
    out = np.concatenate(
        [np.asarray(r["out"], dtype=np.float32) for r in res.results], axis=0
    )
    return out, res


def kernel(a, b):
    out, _ = _run(a, b, trace=False)
    return out
